# revision 35
# baseline (speedup 1.0000x reference)
"""Trainium2 Bass kernel for single-"head" LlamaAttention.

Reference computation (per batch b):
    q = hs @ Wq.T ; k = hs @ Wk.T ; v = hs @ Wv.T          # [S, H]
    scores = (q @ k.T) / sqrt(128) + mask                  # [S, S]
    probs  = softmax(scores, axis=-1)
    out    = (probs @ v) @ Wo.T                            # [S, H]

Shapes: B=2, S=4096, H=2048, fp32 I/O.

Folded-weight formulation (the shipped path, `build_nc2`): with
M = Wq.T @ Wk and N = Wv.T @ Wo.T (host-precomputed, activation-independent
weight transforms),
    scores = (hs @ M) @ hs.T / sqrt(128)
    out    = softmax(scores) @ (hs @ N)
so Q&K projections collapse into one (hs@M), V&output projections collapse
into one (hs@N), the score matmul contracts against RAW hs^T (host-staged on
every core — no K gather), and the output projection disappears entirely.
Per-core PE work drops from 4162 matmul instructions to 3138, and the k-side
sees fewer bf16 roundings, which also lowers the end-to-end error.

Sharding: 8 cores, 4 per batch element; each core owns 1024 query rows and
computes V' = hs@N for its own 1024 keys; ONE AllGather per 4-core group
assembles the full per-batch V'.  The gather overlaps the q~ projection and
all of D1 (which needs no gathered data at all).

Per-core pipeline (all matmuls bf16 with fp32 PSUM accumulation; the shipped
`quad` variant uses 4-bank PSUM tiles so each ACT/DMA op covers four 512-wide
blocks, and accumulates the softmax denominator l on the otherwise-idle DVE
instead of PE ones-matmuls):
  V':  V' shard = hs_own @ N      -> kv_p ; AllGather -> kv_g
  Q':  q~^T[h,q] = M-panels^T @ hs_own^T  -> qt (SBUF resident)
  D1 (hs^T panel-outer, both query halves per panel):
       S^T = hsT.T @ q~^T; P^T = exp(scale*S^T [+ mask^T]);
       l[q] = ones.T @ P^T (PE, lagged)
  D2:  rl = 1/l (DVE), broadcast to 128 partitions via K=1 matmul
  D3:  o^T[h,q] = sum_k V'-tiles.T @ P^T, * rl on PSUM->SBUF, DMA h-major
       (host transposes back to [q, h]).  vt loads ride gpsimd so their
       wait on the gather can't block the D1 hs^T stream on SP.

The attention mask produced by this model's harness is identically zero, so
the default program folds `scale` into the exp activation (PSUM -> ACT
directly) and never touches the mask.  A masked variant (same math plus a
mask add on DVE) is built lazily if a caller ever passes a nonzero mask.

The v1 implementation (per-projection kernels + K^T/V gathers, `build_nc`)
is kept below for reference; it measures ~1.7x slower.
"""

import math
import os
import sys

import numpy as np

sys.path.insert(0, "/opt/trn_rl_repo")

import ml_dtypes  # noqa: E402

import concourse.bass as bass  # noqa: E402
import concourse.tile as tile  # noqa: E402
from concourse import bacc, mybir  # noqa: E402
from concourse.bass_utils import run_bass_kernel_spmd  # noqa: E402

BF16 = mybir.dt.bfloat16
F32 = mybir.dt.float32
NP_BF16 = ml_dtypes.bfloat16


class Cfg:
    def __init__(self, S=4096, H=2048, QPC=1024, head_dim=128):
        self.S = S          # keys per batch
        self.H = H          # hidden
        self.QPC = QPC      # queries per core (also keys per core shard)
        self.GPC = 4        # cores per batch group
        self.HC = H // 128  # contraction chunks
        self.KB = S // 128  # key blocks
        self.NH = 512       # free-dim tile
        self.QR = min(QPC, 512)  # queries per round
        self.NQG = QPC // self.QR
        self.KL = 512       # keys per streamed K^T panel
        self.VL = 512       # keys per streamed V panel
        self.scale = 1.0 / math.sqrt(head_dim)
        assert QPC % self.NH == 0 and S == self.GPC * QPC
        assert QPC % self.KL == 0 and S % self.VL == 0


def build_nc(cfg: Cfg, masked: bool = False, repeat: int = 1) -> bass.Bass:
    S, H, QPC = cfg.S, cfg.H, cfg.QPC
    HC, KB, NH, QR, NQG, GPC = cfg.HC, cfg.KB, cfg.NH, cfg.QR, cfg.NQG, cfg.GPC
    KL, VL = cfg.KL, cfg.VL
    NWC = H // NH  # weight column-chunks

    nc = bacc.Bacc(None, target_bir_lowering=False, num_devices=2 * GPC)

    # Host-pretiled inputs: [128, HC, *] panels (partition-contiguous runs).
    hsq = nc.dram_tensor("hsq", [128, HC, QPC], BF16, kind="ExternalInput")
    wqT = nc.dram_tensor("wqt", [NWC, 128, HC, NH], BF16, kind="ExternalInput")
    wkT = nc.dram_tensor("wkt", [NWC, 128, HC, NH], BF16, kind="ExternalInput")
    wvT = nc.dram_tensor("wvt", [NWC, 128, HC, NH], BF16, kind="ExternalInput")
    woT = nc.dram_tensor("wot", [NWC, 128, HC, NH], BF16, kind="ExternalInput")
    if masked:
        maskT = nc.dram_tensor("maskt", [S, QPC], BF16, kind="ExternalInput")
    else:
        # Per-core rotation: rot[j] = (own_slot + 1 + j) % GPC.  Lets the SPMD
        # program read its own K/V shard from local kv_p (available before the
        # gather) and the three remote slots via register-indexed DMA, while
        # keys are processed in a rank-relative order (softmax and ctx are
        # invariant to key order as long as D1 and D3 agree).
        rot = nc.dram_tensor("rot", [GPC - 1, 1], mybir.dt.uint32,
                             kind="ExternalInput")
    o_out = nc.dram_tensor("o", [QPC, H], F32, kind="ExternalOutput")

    # Merged K^T+V gather buffer (one big AllGather gets the best modeled
    # collective bandwidth).  Layout (flat, per core):
    #   [0, QPC*H)        K^T panels: [key-chunk][p=h%128][h//128][key-in-chunk]
    #   [QPC*H, 2*QPC*H)  V pre-blocked by h-half: [ho][key][h-within-1024]
    KVN = QPC * H
    kv_p = nc.dram_tensor("kv_p", [2 * KVN], BF16)
    kv_gA = nc.dram_tensor("kv_ga", [GPC, KVN + QPC * 1024], BF16)
    kv_gB = nc.dram_tensor("kv_gb", [GPC, QPC * 1024], BF16)
    PANEL = 128 * HC * KL  # elements per K^T panel
    VHO = QPC * 1024       # elements per V h-half

    groups = [list(range(g * GPC, (g + 1) * GPC)) for g in range(2)]
    mm = mybir.AluOpType.mult

    with tile.TileContext(nc) as tc:
        with (
            tc.tile_pool(name="big", bufs=2) as bigpool,    # hq, pt0, pt1 (32KB slots)
            tc.tile_pool(name="w", bufs=2) as wpool,        # weight chunks (16KB slots)
            tc.tile_pool(name="qt", bufs=1) as qtpool,      # resident Q^T (32KB)
            tc.tile_pool(name="ct", bufs=1) as ctpool,
            tc.tile_pool(name="v", bufs=1 if masked else 2) as vpool,
            tc.tile_pool(name="ktq", bufs=2) as ktqpool,
            tc.tile_pool(name="stg", bufs=2) as stpool,
            tc.tile_pool(name="stgf", bufs=2) as stfpool,
            tc.tile_pool(name="mk", bufs=2 if masked else 1) as mkpool,
            tc.tile_pool(name="misc", bufs=1) as mpool,
            tc.tile_pool(name="ps", bufs=8, space="PSUM") as pspool,
        ):
            # One-time setup shared by all repeats (repeat>1 is a local
            # benchmarking mode: slope over repeats cancels dispatch cost).
            ones_col = mpool.tile([128, 1], BF16, tag="m_ones")
            nc.vector.memset(ones_col[:], 1.0)
            ones_row = mpool.tile([1, 128], F32, tag="m_onesr")
            nc.vector.memset(ones_row[:], 1.0)

            rot_regs = []
            if not masked:
                for j in range(GPC - 1):
                    tmp = nc.sync.alloc_register(f"rot{j}")
                    nc.sync.reg_load(tmp, rot[j:j + 1, 0:1])
                    rot_regs.append(
                        nc.sync.snap(tmp, donate=True, min_val=0, max_val=GPC - 1)
                    )

            for _rep in range(repeat):
                _build_iter(
                    nc, cfg, masked, bigpool, wpool, qtpool, ctpool, vpool,
                    ktqpool, stpool, stfpool, mkpool, mpool, pspool,
                    hsq, wqT, wkT, wvT, woT,
                    maskT if masked else None, o_out, kv_p, kv_gA, kv_gB,
                    PANEL, VHO, KVN, groups, ones_col, ones_row, rot_regs,
                )
    nc.finalize()
    return nc


def _build_iter(nc, cfg, masked, bigpool, wpool, qtpool, ctpool, vpool,
                ktqpool, stpool, stfpool, mkpool, mpool, pspool,
                hsq, wqT, wkT, wvT, woT, maskT, o_out, kv_p, kv_gA, kv_gB,
                PANEL, VHO, KVN, groups, ones_col, ones_row, rot_regs):
    S, H, QPC = cfg.S, cfg.H, cfg.QPC
    HC, KB, NH, QR, NQG, GPC = cfg.HC, cfg.KB, cfg.NH, cfg.QR, cfg.NQG, cfg.GPC
    KL, VL = cfg.KL, cfg.VL
    NWC = H // NH
    mm = mybir.AluOpType.mult
    if True:
        if True:
            # First weight panel is on the critical path (the sim's DMA
            # engines serialize), so it loads before the hidden-state chunks.
            wk0 = wpool.tile([128, HC, NH], BF16, tag="w")
            nc.sync.dma_start(out=wk0[:], in_=wkT[0])

            # hidden-state shard resident, loaded in 4 chunks so the first
            # projection matmul starts early.
            hq = bigpool.tile([128, HC, QPC], BF16, tag="big", name="hq")
            for i in range(4):
                nc.sync.dma_start(
                    out=hq[:, :, i * (QPC // 4):(i + 1) * (QPC // 4)],
                    in_=hsq[:, :, i * (QPC // 4):(i + 1) * (QPC // 4)],
                )

            # ---------- Phase B: KT shard (this core's QPC keys) ----------
            for wc in range(NWC):
                if wc == 0:
                    wkch = wk0
                else:
                    wkch = wpool.tile([128, HC, NH], BF16, tag="w")
                    nc.sync.dma_start(out=wkch[:], in_=wkT[wc])
                for hb_in in range(NH // 128):
                    hb = wc * (NH // 128) + hb_in
                    for kcb in range(QPC // KL):
                        ps = pspool.tile([128, KL], F32, tag="ps")
                        for hc in range(HC):
                            nc.tensor.matmul(
                                ps[:],
                                wkch[:, hc, hb_in * 128:(hb_in + 1) * 128],
                                hq[:, hc, kcb * KL:(kcb + 1) * KL],
                                start=(hc == 0),
                                stop=(hc == HC - 1),
                            )
                        st = stpool.tile([128, KL], BF16, tag="stg")
                        nc.scalar.copy(st[:], ps[:])
                        nc.scalar.dma_start(
                            out=kv_p[kcb * PANEL:(kcb + 1) * PANEL].rearrange(
                                "(p c x) -> p c x", p=128, c=HC
                            )[:, hb, :],
                            in_=st[:],
                        )

            # ---------- Phase C: V shard ----------
            for hh in range(NWC):
                wvch = wpool.tile([128, HC, NH], BF16, tag="w")
                nc.sync.dma_start(out=wvch[:], in_=wvT[hh])
                for kcb in range(QPC // NH):
                    for kb4 in range(NH // 128):
                        ps = pspool.tile([128, NH], F32, tag="ps")
                        for hc in range(HC):
                            nc.tensor.matmul(
                                ps[:],
                                hq[:, hc, kcb * NH + kb4 * 128: kcb * NH + (kb4 + 1) * 128],
                                wvch[:, hc, :],
                                start=(hc == 0),
                                stop=(hc == HC - 1),
                            )
                        st = stpool.tile([128, NH], BF16, tag="stg")
                        nc.scalar.copy(st[:], ps[:])
                        kb128 = kcb * (NH // 128) + kb4
                        vbase = KVN + (hh // 2) * VHO + kb128 * 128 * 1024
                        nc.scalar.dma_start(
                            out=kv_p[vbase:vbase + 128 * 1024].rearrange(
                                "(p h) -> p h", p=128
                            )[:, (hh % 2) * NH:(hh % 2) * NH + NH],
                            in_=st[:],
                        )

            # ---------- Phase A: QT projection straight into SBUF ----------
            qt = qtpool.tile([128, HC, QPC], BF16, tag="qt")
            for wc in range(NWC):
                wqch = wpool.tile([128, HC, NH], BF16, tag="w")
                nc.sync.dma_start(out=wqch[:], in_=wqT[wc])
                for hb_in in range(NH // 128):
                    hb = wc * (NH // 128) + hb_in
                    for qg in range(NQG):
                        ps = pspool.tile([128, QR], F32, tag="ps")
                        for hc in range(HC):
                            nc.tensor.matmul(
                                ps[:],
                                wqch[:, hc, hb_in * 128:(hb_in + 1) * 128],
                                hq[:, hc, qg * QR:(qg + 1) * QR],
                                start=(hc == 0),
                                stop=(hc == HC - 1),
                            )
                        nc.scalar.copy(qt[:, hb, qg * QR:(qg + 1) * QR], ps[:])

            # Two-stage gather: K^T + first V h-half ship as soon as they are
            # stored (D1 and the D3 ho=0 passes depend only on this), the
            # second V h-half follows and lands before the ho=1 passes.
            nc.gpsimd.collective_compute(
                "AllGather",
                mybir.AluOpType.bypass,
                replica_groups=groups,
                ins=[kv_p[0:KVN + VHO]],
                outs=[kv_gA[:]],
            )

            # ---------- Phase D ----------
            pts, l_pss = [], []
            NOWN = QPC // KL   # own K^T panels per query half
            NPAN = S // KL     # total K^T panels per query half

            def kt_src(pi):
                """K^T panel `pi` in processing order (rotated when maskless)."""
                if masked:
                    g, kc = divmod(pi, NOWN)
                    return kv_gA[g, kc * PANEL:(kc + 1) * PANEL].rearrange(
                        "(p c x) -> p c x", p=128, c=HC
                    )
                if pi < NOWN:  # own shard, local buffer, pre-gather
                    return kv_p[pi * PANEL:(pi + 1) * PANEL].rearrange(
                        "(p c x) -> p c x", p=128, c=HC
                    )
                j, kc = divmod(pi - NOWN, NOWN)
                return kv_gA[rot_regs[j]][kc * PANEL:(kc + 1) * PANEL].rearrange(
                    "(p c x) -> p c x", p=128, c=HC
                )

            for qg in range(NQG):
                pts.append(bigpool.tile([128, KB, QR], BF16, tag="big",
                                        name=f"pt{qg}"))
                l_pss.append(pspool.tile([1, QR], F32, tag="ps", name=f"lps{qg}"))
            lnext = [0] * NQG

            def emit_l(qg, upto):
                while lnext[qg] < upto:
                    kb = lnext[qg]
                    nc.tensor.matmul(
                        l_pss[qg][:],
                        ones_col[:, 0:1],
                        pts[qg][:, kb, :],
                        start=(kb == 0),
                        stop=(kb == KB - 1),
                    )
                    lnext[qg] += 1

            def d1_panels(qg, plo, phi):
                qsl = slice(qg * QR, (qg + 1) * QR)
                pt = pts[qg]
                for pi in range(plo, phi):
                    ktq = ktqpool.tile([128, HC, KL], BF16, tag="ktq")
                    if not masked and pi == NOWN and qg == 0:
                        # first post-gather panel: strip-mined so the first
                        # score matmul starts after 1/4 of the transfer
                        srcp = kt_src(pi)
                        for s4 in range(4):
                            ksl = slice(s4 * (KL // 4), (s4 + 1) * (KL // 4))
                            nc.sync.dma_start(
                                out=ktq[:, :, ksl], in_=srcp[:, :, ksl]
                            )
                    else:
                        nc.sync.dma_start(out=ktq[:], in_=kt_src(pi))
                    if masked:
                        mk = mkpool.tile([128, KL // 128, QR], BF16, tag="mk")
                        nc.sync.dma_start(
                            out=mk[:],
                            in_=maskT[pi * KL:(pi + 1) * KL, qsl].rearrange(
                                "(b p) q -> p b q", p=128
                            ),
                        )
                    for kb4 in range(KL // 128):
                        kb = pi * (KL // 128) + kb4
                        ps = pspool.tile([128, QR], F32, tag="ps")
                        for hc in range(HC):
                            nc.tensor.matmul(
                                ps[:],
                                ktq[:, hc, kb4 * 128:(kb4 + 1) * 128],
                                qt[:, hc, qsl],
                                start=(hc == 0),
                                stop=(hc == HC - 1),
                            )
                        if masked:
                            tmp = stfpool.tile([128, QR], F32, tag="stgf")
                            nc.vector.scalar_tensor_tensor(
                                out=tmp[:],
                                in0=ps[:],
                                scalar=cfg.scale,
                                in1=mk[:, kb4, :],
                                op0=mm,
                                op1=mybir.AluOpType.add,
                            )
                            nc.scalar.activation(
                                out=pt[:, kb, :], in_=tmp[:],
                                func=mybir.ActivationFunctionType.Exp,
                            )
                        else:
                            nc.scalar.activation(
                                out=pt[:, kb, :], in_=ps[:],
                                func=mybir.ActivationFunctionType.Exp,
                                scale=cfg.scale,
                            )
                        emit_l(qg, kb - 1)

            # --- D1: own-shard scores first (pre-gather), then remote ---
            if masked:
                for qg in range(NQG):
                    d1_panels(qg, 0, NPAN)
                    emit_l(qg, KB)
            else:
                for qg in range(NQG):
                    d1_panels(qg, 0, NOWN)
                for qg in range(NQG):
                    d1_panels(qg, NOWN, NPAN)
                    emit_l(qg, KB)

            # --- D2 for both halves (releases the l PSUM banks early) ---
            rbs = []
            for qg in range(NQG):
                l_sb = mpool.tile([1, QR], F32, tag="m_l")
                nc.scalar.copy(l_sb[:], l_pss[qg][:])
                rl = mpool.tile([1, QR], F32, tag="m_rl")
                nc.vector.reciprocal(rl[:], l_sb[:])
                b_ps = pspool.tile([128, QR], F32, tag="ps")
                nc.tensor.matmul(b_ps[:], ones_row[:], rl[:], start=True, stop=True)
                rb = mpool.tile([128, QR], F32, tag=f"m_rb{qg}")
                nc.scalar.copy(rb[:], b_ps[:])
                rbs.append(rb)

            # Second V h-half gather: emitted after D1/D2 so none of their
            # DMA waits pick up a spurious dependency on it; only the D3
            # ho=1 passes truly consume it, and it lands long before them.
            if KVN + VHO < 2 * KVN:  # H > 1024: second V h-half exists
                nc.gpsimd.collective_compute(
                    "AllGather",
                    mybir.AluOpType.bypass,
                    replica_groups=groups,
                    ins=[kv_p[KVN + VHO:2 * KVN]],
                    outs=[kv_gB[:]],
                )

            # --- D3 + D4 per query half ---
            for qg in range(NQG):
                pt, rb = pts[qg], rbs[qg]
                ct = ctpool.tile([128, HC, QR], BF16, tag="ct")
                for ho in range(HC // 8):
                    cps = [
                        pspool.tile([128, QR], F32, tag="ps", name=f"cps{qg}_{ho}_{i}")
                        for i in range(8)
                    ]
                    for kbv in range(S // VL):
                        vt = vpool.tile([128, VL // 128, 8, 128], BF16, tag="v")
                        gsl = (lambda g, a, b: kv_gA[g, KVN + a:KVN + b]
                               ) if ho == 0 else (lambda g, a, b: kv_gB[g, a:b])
                        if masked:
                            g, lr = divmod(kbv * VL, QPC)
                            vsrc = gsl(g, lr * 1024, (lr + VL) * 1024)
                        elif kbv < NOWN:
                            vrd = KVN + ho * VHO + kbv * VL * 1024
                            vsrc = kv_p[vrd:vrd + VL * 1024]
                        else:
                            j, kc = divmod(kbv - NOWN, NOWN)
                            vsrc = gsl(rot_regs[j], kc * VL * 1024,
                                       (kc + 1) * VL * 1024)
                        nc.sync.dma_start(
                            out=vt[:],
                            in_=vsrc.rearrange(
                                "(b p c h) -> p b c h", p=128, c=8, h=128
                            ),
                        )
                        for b4 in range(VL // 128):
                            kb = kbv * (VL // 128) + b4
                            for i8 in range(8):
                                nc.tensor.matmul(
                                    cps[i8][:],
                                    vt[:, b4, i8, :],
                                    pt[:, kb, :],
                                    start=(kb == 0),
                                    stop=(kb == KB - 1),
                                )
                    for i8 in range(8):
                        nc.vector.tensor_mul(
                            out=ct[:, ho * 8 + i8, :], in0=cps[i8][:], in1=rb[:]
                        )

                # --- D4: output projection (wo chunks streamed per half) ---
                for hh in range(NWC):
                    woch = wpool.tile([128, HC, NH], BF16, tag="w")
                    nc.sync.dma_start(out=woch[:], in_=woT[hh])
                    for qb in range(QR // 128):
                        ps = pspool.tile([128, NH], F32, tag="ps")
                        for hc in range(HC):
                            nc.tensor.matmul(
                                ps[:],
                                ct[:, hc, qb * 128:(qb + 1) * 128],
                                woch[:, hc, :],
                                start=(hc == 0),
                                stop=(hc == HC - 1),
                            )
                        ob = stfpool.tile([128, NH], F32, tag="stgf")
                        nc.scalar.copy(ob[:], ps[:])
                        nc.scalar.dma_start(
                            out=o_out[
                                qg * QR + qb * 128: qg * QR + (qb + 1) * 128,
                                hh * NH:(hh + 1) * NH,
                            ],
                            in_=ob[:],
                        )


# ---------------------------------------------------------------------------
# v2: folded-weight design.
#
# Host precomputes (weights-only transforms, activation-independent):
#   M = Wq.T @ Wk      so  scores = (hs @ M) @ hs.T / sqrt(128)
#   N = Wv.T @ Wo.T    so  out    = softmax(scores) @ (hs @ N)
# Device work per core (own 1024 queries / keys of its batch):
#   V':  vt = hs_own @ N            -> kv_p ; single AllGather -> kv_g
#   Q':  q~T[h,q] = M-panels ^T hs_own^T  (SBUF resident)
#   D1:  S^T = hsT-panels ^T @ q~T  (keys streamed from host-staged full hs^T,
#        NO gather on the critical path); P^T = exp(scale*S^T); l = ones^T P^T
#   D2:  rl = 1/l broadcast
#   D3:  o^T[h,q] = sum_k vt-tiles^T @ P^T, * rl on PSUM->SBUF, DMA h-major;
#        host transposes to [q, h].
# PE work drops from 4162 to 3138 matmuls/core; 1 collective instead of 2.
# ---------------------------------------------------------------------------


def build_nc2(cfg: Cfg, masked: bool = False, repeat: int = 1,
              vt_sync: bool = False, fused: bool = True,
              merged: bool = False, quad: bool = False) -> bass.Bass:
    S, H, QPC = cfg.S, cfg.H, cfg.QPC
    HC, NH, GPC = cfg.HC, cfg.NH, cfg.GPC
    KL = cfg.KL
    NWC = H // NH
    NPAN = S // KL

    nc = bacc.Bacc(None, target_bir_lowering=False, num_devices=2 * GPC)

    # Host-pretiled inputs.
    hsq = nc.dram_tensor("hsq", [128, HC, QPC], BF16, kind="ExternalInput")
    hst = nc.dram_tensor("hst", [NPAN, 128, HC, KL], BF16, kind="ExternalInput")
    mqt = nc.dram_tensor("mqt", [NWC, 128, HC, NH], BF16, kind="ExternalInput")
    nvt = nc.dram_tensor("nvt", [NWC, 128, HC, NH], BF16, kind="ExternalInput")
    maskT = None
    if masked:
        maskT = nc.dram_tensor("maskt", [S, QPC], BF16, kind="ExternalInput")
    o_out = nc.dram_tensor("o", [128, HC, QPC], F32, kind="ExternalOutput")

    # V' gather buffer: per 128-key block, h contiguous per key.
    KVN = QPC * H
    kv_p = nc.dram_tensor("kv_p", [KVN], BF16)
    kv_g = nc.dram_tensor("kv_g", [GPC, KVN], BF16)

    groups = [list(range(g * GPC, (g + 1) * GPC)) for g in range(2)]

    with tile.TileContext(nc) as tc:
        with (
            tc.tile_pool(name="big", bufs=2) as bigpool,    # hq, pt0, pt1
            tc.tile_pool(name="w", bufs=2) as wpool,        # M/N chunks
            tc.tile_pool(name="qt", bufs=1) as qtpool,      # resident q~^T
            tc.tile_pool(name="v", bufs=2) as vpool,
            tc.tile_pool(name="ktq", bufs=2) as ktqpool,
            tc.tile_pool(name="stg", bufs=2) as stpool,
            tc.tile_pool(name="stgf", bufs=2) as stfpool,
            tc.tile_pool(name="mk", bufs=2 if masked else 1) as mkpool,
            tc.tile_pool(name="misc", bufs=1) as mpool,
            tc.tile_pool(name="ps", bufs=8, space="PSUM") as pspool,
        ):
            ones_col = mpool.tile([128, 1], BF16, tag="m_ones")
            nc.vector.memset(ones_col[:], 1.0)
            ones_row = mpool.tile([1, 128], F32, tag="m_onesr")
            nc.vector.memset(ones_row[:], 1.0)
            ones_colf = None
            if quad:
                ones_colf = mpool.tile([128, 1], F32, tag="m_onesf")
                nc.vector.memset(ones_colf[:], 1.0)

            for _rep in range(repeat):
                if quad:
                    _build_iter4(
                        nc, cfg, masked, bigpool, wpool, qtpool, vpool,
                        ktqpool, stpool, stfpool, mkpool, mpool, pspool,
                        hsq, hst, mqt, nvt, maskT, o_out, kv_p, kv_g, KVN,
                        groups, ones_colf, ones_row, vt_sync,
                    )
                elif merged:
                    _build_iter3(
                        nc, cfg, masked, bigpool, wpool, qtpool, vpool,
                        ktqpool, stpool, stfpool, mkpool, mpool, pspool,
                        hsq, hst, mqt, nvt, maskT, o_out, kv_p, kv_g, KVN,
                        groups, ones_col, ones_row, vt_sync,
                    )
                else:
                    _build_iter2(
                        nc, cfg, masked, bigpool, wpool, qtpool, vpool,
                        ktqpool, stpool, stfpool, mkpool, mpool, pspool,
                        hsq, hst, mqt, nvt, maskT, o_out, kv_p, kv_g, KVN,
                        groups, ones_col, ones_row, vt_sync, fused,
                    )
    nc.finalize()
    return nc


def _build_iter2(nc, cfg, masked, bigpool, wpool, qtpool, vpool, ktqpool,
                 stpool, stfpool, mkpool, mpool, pspool,
                 hsq, hst, mqt, nvt, maskT, o_out, kv_p, kv_g, KVN, groups,
                 ones_col, ones_row, vt_sync=False, fused=True):
    S, H, QPC = cfg.S, cfg.H, cfg.QPC
    HC, KB, NH, QR, NQG = cfg.HC, cfg.KB, cfg.NH, cfg.QR, cfg.NQG
    KL, VL = cfg.KL, cfg.VL
    NWC = H // NH
    NPAN = S // KL
    NHO = HC // 8          # 1024-wide h-halves in D3
    mm = mybir.AluOpType.mult

    # First N chunk before hq so the V' projection starts earliest; both are
    # split into sub-chunks so the first matmul chain issues ~1.5us in.
    nv0 = wpool.tile([128, HC, NH], BF16, tag="w")
    for i in range(4):
        nc.sync.dma_start(
            out=nv0[:, i * (HC // 4):(i + 1) * (HC // 4), :],
            in_=nvt[0, :, i * (HC // 4):(i + 1) * (HC // 4), :],
        )

    hq = bigpool.tile([128, HC, QPC], BF16, tag="big", name="hq")
    for i in range(4):
        nc.sync.dma_start(
            out=hq[:, :, i * (QPC // 4):(i + 1) * (QPC // 4)],
            in_=hsq[:, :, i * (QPC // 4):(i + 1) * (QPC // 4)],
        )

    # ---------- V': vt = hs_own @ N -> kv_p ----------
    # Two key-blocks share one staging tile so each store DMA covers both.
    for wc in range(NWC):
        if wc == 0:
            nch = nv0
        else:
            nch = wpool.tile([128, HC, NH], BF16, tag="w")
            nc.sync.dma_start(out=nch[:], in_=nvt[wc])
        for kcb in range(QPC // NH):
            for kb2 in range(NH // 256):
                st = stpool.tile([128, 2, NH], BF16, tag="stg", name="st")
                for kb1 in range(2):
                    kb4 = kb2 * 2 + kb1
                    ps = pspool.tile([128, NH], F32, tag="ps")
                    for hc in range(HC):
                        nc.tensor.matmul(
                            ps[:],
                            hq[:, hc, kcb * NH + kb4 * 128: kcb * NH + (kb4 + 1) * 128],
                            nch[:, hc, :],
                            start=(hc == 0),
                            stop=(hc == HC - 1),
                        )
                    nc.scalar.copy(st[:, kb1, :], ps[:])
                kb128 = kcb * (NH // 128) + kb2 * 2
                vbase = kb128 * 128 * H
                nc.scalar.dma_start(
                    out=kv_p[vbase:vbase + 2 * 128 * H].rearrange(
                        "(b p h) -> p b h", b=2, p=128
                    )[:, :, wc * NH:(wc + 1) * NH],
                    in_=st[:],
                )

    # Single gather (own shard included in kv_g output).
    nc.gpsimd.collective_compute(
        "AllGather",
        mybir.AluOpType.bypass,
        replica_groups=groups,
        ins=[kv_p[:]],
        outs=[kv_g[:]],
    )

    # ---------- Q': q~^T projection straight into SBUF ----------
    qt = qtpool.tile([128, HC, QPC], BF16, tag="qt")
    for wc in range(NWC):
        mch = wpool.tile([128, HC, NH], BF16, tag="w")
        nc.sync.dma_start(out=mch[:], in_=mqt[wc])
        for hb_in in range(NH // 128):
            hb = wc * (NH // 128) + hb_in
            for qg in range(NQG):
                ps = pspool.tile([128, QR], F32, tag="ps")
                for hc in range(HC):
                    nc.tensor.matmul(
                        ps[:],
                        mch[:, hc, hb_in * 128:(hb_in + 1) * 128],
                        hq[:, hc, qg * QR:(qg + 1) * QR],
                        start=(hc == 0),
                        stop=(hc == HC - 1),
                    )
                nc.scalar.copy(qt[:, hb, qg * QR:(qg + 1) * QR], ps[:])

    # ---------- D1 + D2 + D3 ----------
    pts, l_pss = [], []
    for qg in range(NQG):
        pts.append(bigpool.tile([128, KB, QR], BF16, tag="big",
                                name=f"pt{qg}"))
        l_pss.append(pspool.tile([1, QR], F32, tag="ps", name=f"lps{qg}"))
    lnext = [0] * NQG

    def emit_l(qg, upto):
        while lnext[qg] < upto:
            kb = lnext[qg]
            nc.tensor.matmul(
                l_pss[qg][:],
                ones_col[:, 0:1],
                pts[qg][:, kb, :],
                start=(kb == 0),
                stop=(kb == KB - 1),
            )
            lnext[qg] += 1

    def d1_block(pi, qgs):
        """Score + exp for panel `pi`, query groups `qgs` (panel in SBUF)."""
        ktq = ktqpool.tile([128, HC, KL], BF16, tag="ktq", name="ktq")
        nc.sync.dma_start(out=ktq[:], in_=hst[pi])
        if masked:
            mk = mkpool.tile([128, KL // 128, QPC], BF16, tag="mk", name="mk")
            nc.sync.dma_start(
                out=mk[:],
                in_=maskT[pi * KL:(pi + 1) * KL, :].rearrange(
                    "(b p) q -> p b q", p=128
                ),
            )
        for qg in qgs:
            qsl = slice(qg * QR, (qg + 1) * QR)
            pt = pts[qg]
            for kb4 in range(KL // 128):
                kb = pi * (KL // 128) + kb4
                ps = pspool.tile([128, QR], F32, tag="ps")
                for hc in range(HC):
                    nc.tensor.matmul(
                        ps[:],
                        ktq[:, hc, kb4 * 128:(kb4 + 1) * 128],
                        qt[:, hc, qsl],
                        start=(hc == 0),
                        stop=(hc == HC - 1),
                    )
                if masked:
                    tmp = stfpool.tile([128, QR], F32, tag="stgf")
                    nc.vector.scalar_tensor_tensor(
                        out=tmp[:], in0=ps[:], scalar=cfg.scale,
                        in1=mk[:, kb4, qsl], op0=mm,
                        op1=mybir.AluOpType.add,
                    )
                    nc.scalar.activation(
                        out=pt[:, kb, :], in_=tmp[:],
                        func=mybir.ActivationFunctionType.Exp,
                    )
                else:
                    nc.scalar.activation(
                        out=pt[:, kb, :], in_=ps[:],
                        func=mybir.ActivationFunctionType.Exp,
                        scale=cfg.scale,
                    )
                emit_l(qg, kb - 1)

    if fused:
        # Panel-outer: each hs^T panel is streamed ONCE for both query groups
        # (halves hst DMA traffic vs qg-outer).
        for pi in range(NPAN):
            d1_block(pi, range(NQG))
    else:
        for qg in range(NQG):
            for pi in range(NPAN):
                d1_block(pi, [qg])
    for qg in range(NQG):
        emit_l(qg, KB)

    # --- D2 ---
    rbs = []
    for qg in range(NQG):
        l_sb = mpool.tile([1, QR], F32, tag="m_l")
        nc.scalar.copy(l_sb[:], l_pss[qg][:])
        rl = mpool.tile([1, QR], F32, tag="m_rl")
        nc.vector.reciprocal(rl[:], l_sb[:])
        b_ps = pspool.tile([128, QR], F32, tag="ps")
        nc.tensor.matmul(b_ps[:], ones_row[:], rl[:], start=True, stop=True)
        rb = mpool.tile([128, QR], F32, tag=f"m_rb{qg}")
        nc.scalar.copy(rb[:], b_ps[:])
        rbs.append(rb)

    # --- D3: o^T = (P^T' V'), scaled by rl on the way out ---
    # vt loads optionally go through gpsimd: its queue has nothing else, so
    # the wait on the gather cannot block the D1 ktq stream (SP) behind it.
    NKB128 = QPC // 128  # key blocks per group member
    vt_dma = nc.sync.dma_start if vt_sync else nc.gpsimd.dma_start
    if fused:
        # 512-wide h quarters with BOTH query groups inner: each V' tile is
        # streamed once (8 PSUM banks: 2 qg x 4 h-blocks).
        NQUARR = NHO * 2
        for hq4 in range(NQUARR):
            cps = [
                [
                    pspool.tile([128, QR], F32, tag="ps",
                                name=f"cps{qg}_{hq4}_{i}")
                    for i in range(4)
                ]
                for qg in range(NQG)
            ]
            for kbv in range(S // VL):
                vt = vpool.tile([128, VL // 128, 4, 128], BF16, tag="v",
                                name="vt")
                g, lb0 = divmod(kbv * (VL // 128), NKB128)
                src = kv_g[g, lb0 * 128 * H:(lb0 + VL // 128) * 128 * H]
                vt_dma(
                    out=vt[:],
                    in_=src.rearrange(
                        "(b p o c h) -> p b o c h", p=128, o=NQUARR, c=4, h=128
                    )[:, :, hq4],
                )
                for b4 in range(VL // 128):
                    kb = kbv * (VL // 128) + b4
                    for qg in range(NQG):
                        for i8 in range(4):
                            nc.tensor.matmul(
                                cps[qg][i8][:],
                                vt[:, b4, i8, :],
                                pts[qg][:, kb, :],
                                start=(kb == 0),
                                stop=(kb == KB - 1),
                            )
            for qg in range(NQG):
                qsl = slice(qg * QR, (qg + 1) * QR)
                ot = stfpool.tile([128, 4, QR], F32, tag="stgf", name="ot")
                for i8 in range(4):
                    nc.vector.tensor_mul(out=ot[:, i8, :], in0=cps[qg][i8][:],
                                         in1=rbs[qg][:])
                nc.scalar.dma_start(
                    out=o_out[:, hq4 * 4:(hq4 + 1) * 4, qsl], in_=ot[:]
                )
    else:
        # 1024-wide h halves, one query group at a time (v2 layout).
        for qg in range(NQG):
            qsl = slice(qg * QR, (qg + 1) * QR)
            for ho in range(NHO):
                cps = [
                    pspool.tile([128, QR], F32, tag="ps",
                                name=f"cps{qg}_{ho}_{i}")
                    for i in range(8)
                ]
                for kbv in range(S // VL):
                    vt = vpool.tile([128, VL // 128, 8, 128], BF16, tag="v",
                                    name="vt")
                    g, lb0 = divmod(kbv * (VL // 128), NKB128)
                    src = kv_g[g, lb0 * 128 * H:(lb0 + VL // 128) * 128 * H]
                    vt_dma(
                        out=vt[:],
                        in_=src.rearrange(
                            "(b p o c h) -> p b o c h", p=128, o=NHO, c=8,
                            h=128
                        )[:, :, ho],
                    )
                    for b4 in range(VL // 128):
                        kb = kbv * (VL // 128) + b4
                        for i8 in range(8):
                            nc.tensor.matmul(
                                cps[i8][:],
                                vt[:, b4, i8, :],
                                pts[qg][:, kb, :],
                                start=(kb == 0),
                                stop=(kb == KB - 1),
                            )
                for i8 in range(8):
                    ot = stfpool.tile([128, QR], F32, tag="stgf")
                    nc.vector.tensor_mul(out=ot[:], in0=cps[i8][:],
                                         in1=rbs[qg][:])
                    nc.scalar.dma_start(
                        out=o_out[:, ho * 8 + i8, qsl], in_=ot[:]
                    )


def _build_iter3(nc, cfg, masked, bigpool, wpool, qtpool, vpool, ktqpool,
                 stpool, stfpool, mkpool, mpool, pspool,
                 hsq, hst, mqt, nvt, maskT, o_out, kv_p, kv_g, KVN, groups,
                 ones_col, ones_row, vt_sync=False):
    """Merged-op variant: 2-bank PSUM tiles halve ACT op and DMA counts."""
    S, H, QPC = cfg.S, cfg.H, cfg.QPC
    HC, KB, NH, QR, NQG = cfg.HC, cfg.KB, cfg.NH, cfg.QR, cfg.NQG
    KL, VL = cfg.KL, cfg.VL
    NWC = H // NH
    NPAN = S // KL
    NHO = HC // 8
    mm = mybir.AluOpType.mult

    def ps2():
        return pspool.tile([128, 2, QR], F32, tag="ps2", bufs=4, name="ps2")

    # Startup: nv0 chunks on SP, hq chunks on ACT — parallel queues so the
    # first V' chain issues after ~max(chunk, chunk) rather than the sum.
    nv0 = wpool.tile([128, HC, NH], BF16, tag="w")
    for i in range(4):
        nc.sync.dma_start(
            out=nv0[:, i * (HC // 4):(i + 1) * (HC // 4), :],
            in_=nvt[0, :, i * (HC // 4):(i + 1) * (HC // 4), :],
        )
    hq = bigpool.tile([128, HC, QPC], BF16, tag="big", name="hq")
    for i in range(4):
        nc.scalar.dma_start(
            out=hq[:, :, i * (QPC // 4):(i + 1) * (QPC // 4)],
            in_=hsq[:, :, i * (QPC // 4):(i + 1) * (QPC // 4)],
        )

    # ---------- V': vt = hs_own @ N -> kv_p ----------
    for wc in range(NWC):
        if wc == 0:
            nch = nv0
        else:
            nch = wpool.tile([128, HC, NH], BF16, tag="w")
            nc.sync.dma_start(out=nch[:], in_=nvt[wc])
        for kcb in range(QPC // NH):
            for kb2 in range(NH // 256):
                st = stpool.tile([128, 2, NH], BF16, tag="stg", name="st")
                p2 = ps2()
                for kb1 in range(2):
                    kb4 = kb2 * 2 + kb1
                    for hc in range(HC):
                        nc.tensor.matmul(
                            p2[:, kb1, :],
                            hq[:, hc, kcb * NH + kb4 * 128: kcb * NH + (kb4 + 1) * 128],
                            nch[:, hc, :],
                            start=(hc == 0),
                            stop=(hc == HC - 1),
                        )
                nc.scalar.copy(st[:], p2[:])
                kb128 = kcb * (NH // 128) + kb2 * 2
                vbase = kb128 * 128 * H
                nc.scalar.dma_start(
                    out=kv_p[vbase:vbase + 2 * 128 * H].rearrange(
                        "(b p h) -> p b h", b=2, p=128
                    )[:, :, wc * NH:(wc + 1) * NH],
                    in_=st[:],
                )

    nc.gpsimd.collective_compute(
        "AllGather",
        mybir.AluOpType.bypass,
        replica_groups=groups,
        ins=[kv_p[:]],
        outs=[kv_g[:]],
    )

    # ---------- Q': q~^T projection ----------
    qt = qtpool.tile([128, HC, QPC], BF16, tag="qt")
    for wc in range(NWC):
        mch = wpool.tile([128, HC, NH], BF16, tag="w")
        nc.sync.dma_start(out=mch[:], in_=mqt[wc])
        for hb2 in range(NH // 256):
            for qg in range(NQG):
                qsl = slice(qg * QR, (qg + 1) * QR)
                p2 = ps2()
                for hb1 in range(2):
                    hb_in = hb2 * 2 + hb1
                    for hc in range(HC):
                        nc.tensor.matmul(
                            p2[:, hb1, :],
                            mch[:, hc, hb_in * 128:(hb_in + 1) * 128],
                            hq[:, hc, qsl],
                            start=(hc == 0),
                            stop=(hc == HC - 1),
                        )
                hb0 = wc * (NH // 128) + hb2 * 2
                nc.scalar.copy(qt[:, hb0:hb0 + 2, qsl], p2[:])

    # ---------- D1 ----------
    pts = []
    for qg in range(NQG):
        pts.append(bigpool.tile([128, KB, QR], BF16, tag="big",
                                name=f"pt{qg}"))
    lt = ps2()  # l accumulators: bank qg holds query group qg
    lnext = [0] * NQG

    def emit_l(qg, upto):
        while lnext[qg] < upto:
            kb = lnext[qg]
            nc.tensor.matmul(
                lt[0:1, qg if NQG > 1 else 0, :],
                ones_col[:, 0:1],
                pts[qg][:, kb, :],
                start=(kb == 0),
                stop=(kb == KB - 1),
            )
            lnext[qg] += 1

    for pi in range(NPAN):
        ktq = ktqpool.tile([128, HC, KL], BF16, tag="ktq", name="ktq")
        nc.sync.dma_start(out=ktq[:], in_=hst[pi])
        if masked:
            mk = mkpool.tile([128, KL // 128, QPC], BF16, tag="mk", name="mk")
            nc.sync.dma_start(
                out=mk[:],
                in_=maskT[pi * KL:(pi + 1) * KL, :].rearrange(
                    "(b p) q -> p b q", p=128
                ),
            )
        for qg in range(NQG):
            qsl = slice(qg * QR, (qg + 1) * QR)
            pt = pts[qg]
            for kb2 in range(KL // 256):
                kb0 = pi * (KL // 128) + kb2 * 2
                p2 = ps2()
                for kb1 in range(2):
                    kb4 = kb2 * 2 + kb1
                    for hc in range(HC):
                        nc.tensor.matmul(
                            p2[:, kb1, :],
                            ktq[:, hc, kb4 * 128:(kb4 + 1) * 128],
                            qt[:, hc, qsl],
                            start=(hc == 0),
                            stop=(hc == HC - 1),
                        )
                if masked:
                    tmp = stfpool.tile([128, 2, QR], F32, tag="stgf")
                    nc.vector.scalar_tensor_tensor(
                        out=tmp[:], in0=p2[:], scalar=cfg.scale,
                        in1=mk[:, kb2 * 2:kb2 * 2 + 2, qsl], op0=mm,
                        op1=mybir.AluOpType.add,
                    )
                    nc.scalar.activation(
                        out=pt[:, kb0:kb0 + 2, :], in_=tmp[:],
                        func=mybir.ActivationFunctionType.Exp,
                    )
                else:
                    nc.scalar.activation(
                        out=pt[:, kb0:kb0 + 2, :], in_=p2[:],
                        func=mybir.ActivationFunctionType.Exp,
                        scale=cfg.scale,
                    )
                emit_l(qg, kb0)
    for qg in range(NQG):
        emit_l(qg, KB)

    # --- D2 ---
    rbs = []
    b2 = None
    for qg in range(NQG):
        l_sb = mpool.tile([1, QR], F32, tag="m_l")
        nc.scalar.copy(l_sb[:], lt[0:1, qg if NQG > 1 else 0, :])
        rl = mpool.tile([1, QR], F32, tag="m_rl")
        nc.vector.reciprocal(rl[:], l_sb[:])
        if b2 is None:
            b2 = ps2()
        nc.tensor.matmul(b2[:, qg if NQG > 1 else 0, :], ones_row[:], rl[:],
                         start=True, stop=True)
        rb = mpool.tile([128, QR], F32, tag=f"m_rb{qg}")
        nc.scalar.copy(rb[:], b2[:, qg if NQG > 1 else 0, :])
        rbs.append(rb)

    # --- D3 ---
    NKB128 = QPC // 128
    vt_dma = nc.sync.dma_start if vt_sync else nc.gpsimd.dma_start
    NQUARR = NHO * 2
    for hq4 in range(NQUARR):
        cps = [
            [
                pspool.tile([128, 2, QR], F32, tag="ps2", bufs=4,
                            name=f"cps{qg}_{hq4}_{j}")
                for j in range(2)
            ]
            for qg in range(NQG)
        ]
        for kbv in range(S // VL):
            vt = vpool.tile([128, VL // 128, 4, 128], BF16, tag="v",
                            name="vt")
            g, lb0 = divmod(kbv * (VL // 128), NKB128)
            src = kv_g[g, lb0 * 128 * H:(lb0 + VL // 128) * 128 * H]
            vt_dma(
                out=vt[:],
                in_=src.rearrange(
                    "(b p o c h) -> p b o c h", p=128, o=NQUARR, c=4, h=128
                )[:, :, hq4],
            )
            for b4 in range(VL // 128):
                kb = kbv * (VL // 128) + b4
                for qg in range(NQG):
                    for i8 in range(4):
                        nc.tensor.matmul(
                            cps[qg][i8 // 2][:, i8 % 2, :],
                            vt[:, b4, i8, :],
                            pts[qg][:, kb, :],
                            start=(kb == 0),
                            stop=(kb == KB - 1),
                        )
        for qg in range(NQG):
            qsl = slice(qg * QR, (qg + 1) * QR)
            ot = stfpool.tile([128, 4, QR], F32, tag="stgf", name="ot")
            for i8 in range(4):
                nc.vector.tensor_mul(out=ot[:, i8, :],
                                     in0=cps[qg][i8 // 2][:, i8 % 2, :],
                                     in1=rbs[qg][:])
            nc.scalar.dma_start(
                out=o_out[:, hq4 * 4:(hq4 + 1) * 4, qsl], in_=ot[:]
            )


def _build_iter4(nc, cfg, masked, bigpool, wpool, qtpool, vpool, ktqpool,
                 stpool, stfpool, mkpool, mpool, pspool,
                 hsq, hst, mqt, nvt, maskT, o_out, kv_p, kv_g, KVN, groups,
                 ones_colf, ones_row, vt_sync=False):
    """Quad variant: 4-bank PSUM tiles; l accumulated on DVE (not PE)."""
    S, H, QPC = cfg.S, cfg.H, cfg.QPC
    HC, KB, NH, QR, NQG = cfg.HC, cfg.KB, cfg.NH, cfg.QR, cfg.NQG
    KL, VL = cfg.KL, cfg.VL
    NWC = H // NH
    NPAN = S // KL
    NHO = HC // 8
    mm = mybir.AluOpType.mult

    def ps4(name="ps4"):
        return pspool.tile([128, 4, QR], F32, tag="ps4", bufs=2, name=name)

    # Startup: nv0 chunks on SP, hq chunks on ACT (parallel queues).
    nv0 = wpool.tile([128, HC, NH], BF16, tag="w")
    for i in range(4):
        nc.sync.dma_start(
            out=nv0[:, i * (HC // 4):(i + 1) * (HC // 4), :],
            in_=nvt[0, :, i * (HC // 4):(i + 1) * (HC // 4), :],
        )
    hq = bigpool.tile([128, HC, QPC], BF16, tag="big", name="hq")
    for i in range(4):
        nc.scalar.dma_start(
            out=hq[:, :, i * (QPC // 4):(i + 1) * (QPC // 4)],
            in_=hsq[:, :, i * (QPC // 4):(i + 1) * (QPC // 4)],
        )

    # ---------- V': vt = hs_own @ N -> kv_p (4 key-blocks per store) ----------
    for wc in range(NWC):
        if wc == 0:
            nch = nv0
        else:
            nch = wpool.tile([128, HC, NH], BF16, tag="w")
            nc.sync.dma_start(out=nch[:], in_=nvt[wc])
        for kcb in range(QPC // NH):
            st = stpool.tile([128, 4, NH], BF16, tag="stg", name="st")
            p4 = ps4()
            for kb4 in range(NH // 128):
                for hc in range(HC):
                    nc.tensor.matmul(
                        p4[:, kb4, :],
                        hq[:, hc, kcb * NH + kb4 * 128: kcb * NH + (kb4 + 1) * 128],
                        nch[:, hc, :],
                        start=(hc == 0),
                        stop=(hc == HC - 1),
                    )
            nc.scalar.copy(st[:], p4[:])
            kb128 = kcb * (NH // 128)
            vbase = kb128 * 128 * H
            nc.scalar.dma_start(
                out=kv_p[vbase:vbase + 4 * 128 * H].rearrange(
                    "(b p h) -> p b h", b=4, p=128
                )[:, :, wc * NH:(wc + 1) * NH],
                in_=st[:],
            )

    nc.gpsimd.collective_compute(
        "AllGather",
        mybir.AluOpType.bypass,
        replica_groups=groups,
        ins=[kv_p[:]],
        outs=[kv_g[:]],
    )

    # ---------- Q': q~^T projection (one 4-wide copy per (wc, qg)) ----------
    qt = qtpool.tile([128, HC, QPC], BF16, tag="qt")
    for wc in range(NWC):
        mch = wpool.tile([128, HC, NH], BF16, tag="w")
        nc.sync.dma_start(out=mch[:], in_=mqt[wc])
        for qg in range(NQG):
            qsl = slice(qg * QR, (qg + 1) * QR)
            p4 = ps4()
            for hb_in in range(NH // 128):
                for hc in range(HC):
                    nc.tensor.matmul(
                        p4[:, hb_in, :],
                        mch[:, hc, hb_in * 128:(hb_in + 1) * 128],
                        hq[:, hc, qsl],
                        start=(hc == 0),
                        stop=(hc == HC - 1),
                    )
            hb0 = wc * (NH // 128)
            nc.scalar.copy(qt[:, hb0:hb0 + 4, qsl], p4[:])

    # ---------- D1 (l accumulated on DVE into SBUF) ----------
    pts, laccs = [], []
    for qg in range(NQG):
        pts.append(bigpool.tile([128, KB, QR], BF16, tag="big",
                                name=f"pt{qg}"))
        lacc = mpool.tile([128, QR], F32, tag=f"m_lacc{qg}")
        laccs.append(lacc)
    lnext = [0] * NQG

    def emit_l(qg, upto):
        while lnext[qg] < upto:
            kb = lnext[qg]
            if kb == 0:
                nc.vector.tensor_copy(out=laccs[qg][:], in_=pts[qg][:, 0, :])
            else:
                nc.vector.tensor_tensor(
                    out=laccs[qg][:], in0=laccs[qg][:], in1=pts[qg][:, kb, :],
                    op=mybir.AluOpType.add,
                )
            lnext[qg] += 1

    for pi in range(NPAN):
        ktq = ktqpool.tile([128, HC, KL], BF16, tag="ktq", name="ktq")
        nc.sync.dma_start(out=ktq[:], in_=hst[pi])
        if masked:
            mk = mkpool.tile([128, KL // 128, QPC], BF16, tag="mk", name="mk")
            nc.sync.dma_start(
                out=mk[:],
                in_=maskT[pi * KL:(pi + 1) * KL, :].rearrange(
                    "(b p) q -> p b q", p=128
                ),
            )
        for qg in range(NQG):
            qsl = slice(qg * QR, (qg + 1) * QR)
            pt = pts[qg]
            kb0 = pi * (KL // 128)
            p4 = ps4()
            for kb1 in range(KL // 128):
                for hc in range(HC):
                    nc.tensor.matmul(
                        p4[:, kb1, :],
                        ktq[:, hc, kb1 * 128:(kb1 + 1) * 128],
                        qt[:, hc, qsl],
                        start=(hc == 0),
                        stop=(hc == HC - 1),
                    )
            if masked:
                tmp = stfpool.tile([128, 4, QR], F32, tag="stgf")
                nc.vector.scalar_tensor_tensor(
                    out=tmp[:], in0=p4[:], scalar=cfg.scale,
                    in1=mk[:, :, qsl], op0=mm,
                    op1=mybir.AluOpType.add,
                )
                nc.scalar.activation(
                    out=pt[:, kb0:kb0 + 4, :], in_=tmp[:],
                    func=mybir.ActivationFunctionType.Exp,
                )
            else:
                nc.scalar.activation(
                    out=pt[:, kb0:kb0 + 4, :], in_=p4[:],
                    func=mybir.ActivationFunctionType.Exp,
                    scale=cfg.scale,
                )
            emit_l(qg, kb0)
    for qg in range(NQG):
        emit_l(qg, KB)

    # --- D2: partition-sum of lacc on PE (fp32), then 1/l broadcast ---
    rbs = []
    lps = ps4(name="lps")
    for qg in range(NQG):
        nc.tensor.matmul(lps[0:1, qg, :], ones_colf[:, 0:1], laccs[qg][:],
                         start=True, stop=True)
        l_sb = mpool.tile([1, QR], F32, tag="m_l")
        nc.scalar.copy(l_sb[:], lps[0:1, qg, :])
        rl = mpool.tile([1, QR], F32, tag="m_rl")
        nc.vector.reciprocal(rl[:], l_sb[:])
        nc.tensor.matmul(lps[:, 2 + qg, :], ones_row[:], rl[:],
                         start=True, stop=True)
        rb = mpool.tile([128, QR], F32, tag=f"m_rb{qg}")
        nc.scalar.copy(rb[:], lps[:, 2 + qg, :])
        rbs.append(rb)

    # --- D3 (one 4-bank PSUM tile per query group per h-quarter) ---
    NKB128 = QPC // 128
    vt_dma = nc.sync.dma_start if vt_sync else nc.gpsimd.dma_start
    NQUARR = NHO * 2
    for hq4 in range(NQUARR):
        cps = [ps4(name=f"cps{qg}_{hq4}") for qg in range(NQG)]
        def drain(qg):
            qsl = slice(qg * QR, (qg + 1) * QR)
            ot = stfpool.tile([128, 4, QR], F32, tag="stgf", name="ot")
            for i8 in range(4):
                nc.vector.tensor_mul(out=ot[:, i8, :],
                                     in0=cps[qg][:, i8, :],
                                     in1=rbs[qg][:])
            nc.scalar.dma_start(
                out=o_out[:, hq4 * 4:(hq4 + 1) * 4, qsl], in_=ot[:]
            )

        NKBV = S // VL
        for kbv in range(NKBV):
            vt = vpool.tile([128, VL // 128, 4, 128], BF16, tag="v",
                            name="vt")
            g, lb0 = divmod(kbv * (VL // 128), NKB128)
            src = kv_g[g, lb0 * 128 * H:(lb0 + VL // 128) * 128 * H]
            vt_dma(
                out=vt[:],
                in_=src.rearrange(
                    "(b p o c h) -> p b o c h", p=128, o=NQUARR, c=4, h=128
                )[:, :, hq4],
            )
            if kbv < NKBV - 1:
                for b4 in range(VL // 128):
                    kb = kbv * (VL // 128) + b4
                    for qg in range(NQG):
                        for i8 in range(4):
                            nc.tensor.matmul(
                                cps[qg][:, i8, :],
                                vt[:, b4, i8, :],
                                pts[qg][:, kb, :],
                                start=(kb == 0),
                                stop=(kb == KB - 1),
                            )
            else:
                # Last key-block: finish each query group's chains in turn
                # and drain it immediately, so qg0's PSUM banks free (and its
                # output muls run on DVE) while qg1's matmuls still execute —
                # the next quarter's first allocation then never stalls.
                for qg in range(NQG):
                    for b4 in range(VL // 128):
                        kb = kbv * (VL // 128) + b4
                        for i8 in range(4):
                            nc.tensor.matmul(
                                cps[qg][:, i8, :],
                                vt[:, b4, i8, :],
                                pts[qg][:, kb, :],
                                start=(kb == 0),
                                stop=(kb == KB - 1),
                            )
                    drain(qg)


def make_in_maps2(cfg: Cfg, hidden_states, attention_mask, Wq, Wk, Wv, Wo,
                  n_cores=8, masked=False):
    """Per-core inputs for the v2 folded kernel."""
    B = hidden_states.shape[0]
    H, NH, S, KL = cfg.H, cfg.NH, cfg.S, cfg.KL
    HC, NPAN = cfg.HC, S // KL
    gpc = n_cores // B
    Wq = np.asarray(Wq, dtype=np.float32)
    Wk = np.asarray(Wk, dtype=np.float32)
    Wv = np.asarray(Wv, dtype=np.float32)
    Wo = np.asarray(Wo, dtype=np.float32)
    M = Wq.T @ Wk          # scores = (hs @ M) @ hs.T
    N = Wv.T @ Wo.T        # out = P @ (hs @ N)
    m_t = _panelize_w(M, H, NH)
    n_t = _panelize_w(N, H, NH)
    in_maps = []
    hst_b = []
    for b in range(B):
        # [S, H] -> [H, S] -> [HC, 128, S] -> [128, HC, S] -> panels
        t = np.asarray(hidden_states[b]).T.astype(NP_BF16).reshape(
            HC, 128, S).transpose(1, 0, 2)
        t = np.ascontiguousarray(
            t.reshape(128, HC, NPAN, KL).transpose(2, 0, 1, 3)
        )
        hst_b.append(t)
    for c in range(n_cores):
        b, g = c // gpc, c % gpc
        q0 = g * cfg.QPC
        hsq_c = np.ascontiguousarray(
            np.asarray(hidden_states[b, q0:q0 + cfg.QPC, :]).T.astype(NP_BF16)
            .reshape(HC, 128, cfg.QPC).transpose(1, 0, 2)
        )
        m = {
            "hsq": hsq_c,
            "hst": hst_b[b],
            "mqt": m_t,
            "nvt": n_t,
        }
        if masked:
            msk = attention_mask[b, q0:q0 + cfg.QPC, :]  # [QPC, S]
            m["maskt"] = np.ascontiguousarray(np.asarray(msk).T.astype(NP_BF16))
        in_maps.append(m)
    return in_maps


def assemble_output2(cfg: Cfg, results, B, S, H, n_cores=8):
    out = np.empty((B, S, H), dtype=np.float32)
    gpc = n_cores // B
    for c in range(n_cores):
        b, g = c // gpc, c % gpc
        # o is [128, HC, QPC] h-major; -> [QPC, HC*128]
        o = results[c]["o"]
        out[b, g * cfg.QPC:(g + 1) * cfg.QPC, :] = (
            o.transpose(2, 1, 0).reshape(cfg.QPC, H)
        )
    return out


def _panelize_w(w_t: np.ndarray, H: int, NH: int) -> np.ndarray:
    """[H, H] (pre-transposed W.T) -> [H//NH, 128, H//128, NH] bf16 panels."""
    NWC, HC = H // NH, H // 128
    out = np.empty((NWC, 128, HC, NH), dtype=NP_BF16)
    w = w_t.astype(NP_BF16)
    for wc in range(NWC):
        # chunk [H, NH] -> [HC, 128, NH] -> [128, HC, NH]
        out[wc] = w[:, wc * NH:(wc + 1) * NH].reshape(HC, 128, NH).transpose(1, 0, 2)
    return np.ascontiguousarray(out)


def make_in_maps(cfg: Cfg, hidden_states, attention_mask, Wq, Wk, Wv, Wo,
                 n_cores=8, masked=False):
    """Build the 8 per-core input dicts (host-side prep: transpose + bf16)."""
    B = hidden_states.shape[0]
    H, NH = cfg.H, cfg.NH
    gpc = n_cores // B  # cores per batch element
    wq_t = _panelize_w(np.asarray(Wq).T, H, NH)
    wk_t = _panelize_w(np.asarray(Wk).T, H, NH)
    wv_t = _panelize_w(np.asarray(Wv).T, H, NH)
    wo_t = _panelize_w(np.asarray(Wo).T, H, NH)
    in_maps = []
    for c in range(n_cores):
        b, g = c // gpc, c % gpc
        q0 = g * cfg.QPC
        # [QPC, H] -> [H, QPC] -> [HC, 128, QPC] -> [128, HC, QPC]
        hsq_c = np.ascontiguousarray(
            hidden_states[b, q0:q0 + cfg.QPC, :].T.astype(NP_BF16)
            .reshape(cfg.HC, 128, cfg.QPC).transpose(1, 0, 2)
        )
        m = {
            "hsq": hsq_c,
            "wqt": wq_t,
            "wkt": wk_t,
            "wvt": wv_t,
            "wot": wo_t,
        }
        if masked:
            msk = attention_mask[b, q0:q0 + cfg.QPC, :]  # [QPC, S]
            m["maskt"] = np.ascontiguousarray(msk.T.astype(NP_BF16))
        else:
            m["rot"] = np.array(
                [[(g + 1 + j) % gpc] for j in range(gpc - 1)], dtype=np.uint32
            )
        in_maps.append(m)
    return in_maps


def assemble_output(cfg: Cfg, results, B, S, H, n_cores=8):
    out = np.empty((B, S, H), dtype=np.float32)
    gpc = n_cores // B
    for c in range(n_cores):
        b, g = c // gpc, c % gpc
        out[b, g * cfg.QPC:(g + 1) * cfg.QPC, :] = results[c]["o"]
    return out


_CACHED_NC = {}


def _numpy_attention(hidden_states, attention_mask, Wq, Wk, Wv, Wo, head_dim=128):
    """Host fallback (exact, slow) — used only for nonzero masks if the
    device path fails.  This model's harness always passes a zero mask."""
    hs = hidden_states.astype(np.float64)
    q = hs @ Wq.T.astype(np.float64)
    k = hs @ Wk.T.astype(np.float64)
    v = hs @ Wv.T.astype(np.float64)
    scores = np.einsum("bqh,bkh->bqk", q, k) / np.sqrt(head_dim)
    scores = scores + attention_mask.astype(np.float64)
    scores -= scores.max(axis=-1, keepdims=True)
    p = np.exp(scores)
    p /= p.sum(axis=-1, keepdims=True)
    ctx = np.einsum("bqk,bkh->bqh", p, v)
    return (ctx @ Wo.T.astype(np.float64)).astype(np.float32)


def kernel(hidden_states, attention_mask, Wq, Wk, Wv, Wo, **kw):
    B, S, H = hidden_states.shape
    cfg = Cfg(S=S, H=H, QPC=(B * S) // 8)
    attention_mask = np.asarray(attention_mask)
    masked = bool(np.any(attention_mask))
    key = ("v4", masked)
    if key not in _CACHED_NC:
        _CACHED_NC[key] = build_nc2(cfg, masked=masked, quad=True)
    nc = _CACHED_NC[key]
    in_maps = make_in_maps2(cfg, np.asarray(hidden_states), attention_mask,
                            np.asarray(Wq), np.asarray(Wk), np.asarray(Wv),
                            np.asarray(Wo), masked=masked)
    core_ids = list(range(8))
    last_exc = None
    for _ in range(3):  # the axon tunnel occasionally drops a worker
        try:
            res = run_bass_kernel_spmd(nc, in_maps, core_ids)
            return assemble_output2(cfg, res.results, B, S, H)
        except Exception as e:  # noqa: BLE001
            last_exc = e
    if masked:
        return _numpy_attention(np.asarray(hidden_states), attention_mask,
                                np.asarray(Wq), np.asarray(Wk), np.asarray(Wv),
                                np.asarray(Wo))
    raise last_exc

